# revision 34
# baseline (speedup 1.0000x reference)
"""ELPH edge-aware GNN message passing on 8 Trainium2 NeuronCores.

Strategy (edge-parallel, dst-sharded, degree-padded slot layout):
  - Core c owns nodes [c*12500, (c+1)*12500) and all edges pointing at them,
    so the scatter-add needs no collective.
  - Key algebraic move: since W2 is applied after the relu and aggregation is
    linear, aggregate relu(hidden) per *node* first and apply W2 to the 12.5K
    aggregates instead of the 100K per-edge messages.
  - Host lays edges out in a degree-padded node-major slot stream: node with
    degree-rank k owns degprof[k] contiguous slot columns (degprof = max over
    cores of the k-th sorted degree, so all cores share one program). The
    scatter-add then becomes a grouped free-axis tensor_reduce on the vector
    engine - no one-hot aggregation matrix, no per-chunk PE matmuls.
  - Hidden layer runs as N=512 moving-dim matmuls with stationary weights
    (vs per-128-edge matmuls): ~900 PE instructions vs ~5900 in the previous
    version, whose runtime was dominated by LDWEIGHTS/instruction churn.
  - relu on the scalar (Act) engine, grouped reduce + bias adds on DVE,
    update MLP woven between message tiles as node ranges complete.
"""
import numpy as np
import ml_dtypes

import concourse.bass as bass
import concourse.mybir as mybir
import concourse.tile as tile
from concourse import bacc
from concourse.bass_utils import run_bass_kernel_spmd

N_NODES = 100000
D_NODE = 64
D_EDGE = 4
H_MSG = 128
H_UPD = 128
N_CORES = 8
N_CORE = N_NODES // N_CORES          # 12500
NPAD = 12800                         # 25 x 512 update supertiles
UPD = 512                            # nodes per update supertile
N_UPD = NPAD // UPD                  # 25
TILE = 1024                          # slots per compute tile (2 PSUM banks)
DMA_TILES = 4                        # compute tiles per DMA block

BF16 = mybir.dt.bfloat16
F32 = mybir.dt.float32
nbf16 = ml_dtypes.bfloat16


def _install_trace_hook_if_possible():
    """Best-effort antenv.axon_hooks shim; only matters if BASS_TRACE is set."""
    import sys
    import types
    try:
        import antenv
        import antenv.axon_hooks  # noqa: F401
        return
    except Exception:
        pass
    try:
        import antenv
        from trn_agent_boot.trn_boot import _ntff_profile_via_ctypes
        mod = types.ModuleType("antenv.axon_hooks")
        mod._hook = _ntff_profile_via_ctypes("/opt/axon/libaxon_pjrt.so")
        mod.set_axon_ntff_profile_hook = lambda h: setattr(mod, "_hook", h)
        mod.get_axon_ntff_profile_hook = lambda: mod._hook
        sys.modules["antenv.axon_hooks"] = mod
        antenv.axon_hooks = mod
    except Exception:
        import os
        os.environ["BASS_NEVER_TRACE"] = "1"


def _build_schedule(degpad_full):
    """Pack degree-padded node groups into 1024-slot tiles.

    Returns (tiles, slot_base, S) where tiles[t] is a list of segments
    (s0_in_tile, rank0, n_nodes, g) of uniform group size g, and
    slot_base[k] is the global slot of rank k's first slot.
    """
    tiles = []
    cur = []          # segments of current tile
    fill = 0
    slot_base = np.zeros(NPAD, dtype=np.int64)
    for k in range(NPAD):
        g = int(degpad_full[k])
        if g > TILE - fill:
            tiles.append(cur)
            cur = []
            fill = 0
        slot_base[k] = len(tiles) * TILE + fill
        if cur and cur[-1][3] == g and cur[-1][0] + cur[-1][2] * g == fill:
            s0, r0, n, _ = cur[-1]
            cur[-1] = (s0, r0, n + 1, g)
        else:
            cur.append((fill, k, 1, g))
        fill += g
        if fill == TILE:
            tiles.append(cur)
            cur = []
            fill = 0
    if cur:
        tiles.append(cur)
    S = len(tiles) * TILE
    return tiles, slot_base, S


def _build_program(tiles, S):
    nc = bacc.Bacc("TRN2", target_bir_lowering=False, debug=False)

    xsd = nc.declare_dram_parameter("xsd", [128, S], BF16, isOutput=False)
    eft = nc.declare_dram_parameter("eft", [D_EDGE + 1, S], BF16, isOutput=False)
    xt = nc.declare_dram_parameter("xt", [D_NODE, NPAD], BF16, isOutput=False)
    w1ab = nc.declare_dram_parameter("w1ab", [128, H_MSG], BF16, isOutput=False)
    w1ca = nc.declare_dram_parameter("w1ca", [D_EDGE + 1, H_MSG], BF16, isOutput=False)
    w2 = nc.declare_dram_parameter("w2", [H_MSG, D_NODE], BF16, isOutput=False)
    u1 = nc.declare_dram_parameter("u1", [2 * D_NODE, H_UPD], BF16, isOutput=False)
    u2 = nc.declare_dram_parameter("u2", [H_UPD, D_NODE], BF16, isOutput=False)
    b2deg = nc.declare_dram_parameter("b2deg", [D_NODE, NPAD], BF16, isOutput=False)
    bu1c = nc.declare_dram_parameter("bu1c", [H_UPD, 1], F32, isOutput=False)
    bu2c = nc.declare_dram_parameter("bu2c", [D_NODE, 1], F32, isOutput=False)
    outt = nc.declare_dram_parameter("outt", [D_NODE, NPAD], F32, isOutput=True)
    warm_out = nc.declare_dram_parameter("warm_out", [128, 8], F32, isOutput=True)

    T = len(tiles)
    # ranks fully reduced after tile t
    cov = np.zeros(T, dtype=np.int64)
    c = 0
    for t in range(T):
        for (_, r0, n, _) in tiles[t]:
            c = max(c, r0 + n)
        cov[t] = c

    Relu = mybir.ActivationFunctionType.Relu
    Copy = mybir.ActivationFunctionType.Copy

    with tile.TileContext(nc) as tc:
        with (
            tc.tile_pool(name="const", bufs=1) as cpool,
            tc.tile_pool(name="xsdb", bufs=4) as xsd_pool,
            tc.tile_pool(name="efb", bufs=3) as ef_pool,
            tc.tile_pool(name="hh", bufs=6) as h_pool,
            tc.tile_pool(name="upd", bufs=3) as upd_pool,
            tc.tile_pool(name="oo", bufs=3) as o_pool,
        ):
            def cload(shape, dt_, param):
                t_ = cpool.tile(shape, dt_, tag=param.name)
                nc.sync.dma_start(out=t_[:], in_=param[:])
                return t_

            w1ab_sb = cload([128, H_MSG], BF16, w1ab)
            w1ca_sb = cload([D_EDGE + 1, H_MSG], BF16, w1ca)

            hagg = cpool.tile([H_MSG, NPAD], BF16, tag="hagg")

            warmo = upd_pool.tile([128, 8], F32, tag="warmo")
            nc.gpsimd.memset(warmo[:], 0)
            nc.sync.dma_start(out=warm_out[:], in_=warmo[:])

            late_consts = {}

            def load_late_consts():
                late_consts["w2"] = cload([H_MSG, D_NODE], BF16, w2)
                late_consts["u1"] = cload([2 * D_NODE, H_UPD], BF16, u1)
                late_consts["u2"] = cload([H_UPD, D_NODE], BF16, u2)
                late_consts["b2deg"] = cload([D_NODE, NPAD], BF16, b2deg)
                late_consts["bu1"] = cload([H_UPD, 1], F32, bu1c)
                late_consts["bu2"] = cload([D_NODE, 1], F32, bu2c)

            XB = 2                       # tiles per xsd DMA
            # upd supertile j emittable once tile upd_tile[j] is processed
            cov2 = {}
            k = 0
            for t in range(T):
                for (_, r0, n, _) in tiles[t]:
                    k = max(k, r0 + n)
                cov2[t] = k
            upd_tile = {}
            for t in range(T):
                while len(upd_tile) < N_UPD and cov2[t] >= (
                        len(upd_tile) + 1) * UPD:
                    upd_tile[len(upd_tile)] = min(t + 6, T - 1)

            def emit_upd(j, pu_pool):
                cols = slice(j * UPD, (j + 1) * UPD)
                pp = pu_pool.tile([128, UPD], F32, space="PSUM", tag="pagpo")
                p_ag = pp[0:D_NODE, :]
                p_o = pp[D_NODE:128, :]
                nc.tensor.matmul(out=p_ag, lhsT=late_consts["w2"][:],
                                 rhs=hagg[:, cols], start=True, stop=True)
                updin = upd_pool.tile([128, UPD], BF16, tag="updin")
                nc.sync.dma_start(out=updin[0:D_NODE, :], in_=xt[:, cols])
                nc.vector.scalar_tensor_tensor(
                    out=updin[D_NODE:128, :], in0=p_ag, scalar=0.0,
                    in1=late_consts["b2deg"][:, cols],
                    op0=mybir.AluOpType.bypass, op1=mybir.AluOpType.add)
                p_u = pu_pool.tile([H_UPD, UPD], F32, space="PSUM", tag="pu")
                nc.tensor.matmul(out=p_u[:], lhsT=late_consts["u1"][:],
                                 rhs=updin[:], start=True, stop=True)
                ru = upd_pool.tile([H_UPD, UPD], BF16, tag="ru")
                nc.scalar.activation(out=ru[:], in_=p_u[:], func=Relu,
                                     bias=late_consts["bu1"][:, 0:1])
                nc.tensor.matmul(out=p_o, lhsT=late_consts["u2"][:], rhs=ru[:],
                                 start=True, stop=True)
                o_sb = o_pool.tile([D_NODE, UPD], F32, tag="osb")
                nc.vector.tensor_scalar(out=o_sb[:], in0=p_o,
                                        scalar1=late_consts["bu2"][:, 0:1],
                                        scalar2=None,
                                        op0=mybir.AluOpType.add)
                nc.sync.dma_start(out=outt[:, cols], in_=o_sb[:])

            next_upd = 0
            with nc.allow_low_precision(reason="bf16 group-reduce aggregation"):
                with (
                    tc.tile_pool(name="ph", bufs=3, space="PSUM") as ph_pool,
                    tc.tile_pool(name="pu", bufs=1, space="PSUM") as pu_pool,
                ):
                    for t in range(T):
                        if t % XB == 0:
                            w = min(XB, T - t) * TILE
                            xsd_sb = xsd_pool.tile([128, XB * TILE], BF16,
                                                   tag="xsd")
                            nc.sync.dma_start(
                                out=xsd_sb[:, 0:w],
                                in_=xsd[:, t * TILE:t * TILE + w])
                        if t % DMA_TILES == 0:
                            w = min(DMA_TILES, T - t) * TILE
                            ef_sb = ef_pool.tile(
                                [D_EDGE + 1, DMA_TILES * TILE], BF16, tag="ef")
                            nc.sync.dma_start(
                                out=ef_sb[:, 0:w],
                                in_=eft[:, t * TILE:t * TILE + w])
                        if t == 2:
                            load_late_consts()
                        if t % 2 == 0:
                            # pair-interleave: all w1ab matmuls for tiles
                            # (t, t+1) back-to-back, then all w1ca - the PE
                            # pipelines same-weight matmuls at stream rate
                            pair = [t] if t + 1 >= T else [t, t + 1]
                            phs = {}
                            for tp in pair:
                                phs[tp] = ph_pool.tile([H_MSG, TILE], F32,
                                                       space="PSUM", tag="ph",
                                                       name=f"ph{tp % 2}")
                            for tp in pair:
                                xo = (tp % XB) * TILE
                                for hh in (0, 512):
                                    nc.tensor.matmul(
                                        out=phs[tp][:, hh:hh + 512],
                                        lhsT=w1ab_sb[:],
                                        rhs=xsd_sb[:, xo + hh:xo + hh + 512],
                                        start=True, stop=False)
                            for tp in pair:
                                eo = (tp % DMA_TILES) * TILE
                                for hh in (0, 512):
                                    nc.tensor.matmul(
                                        out=phs[tp][:, hh:hh + 512],
                                        lhsT=w1ca_sb[:],
                                        rhs=ef_sb[:, eo + hh:eo + hh + 512],
                                        start=False, stop=True)
                            pair_phs = phs
                        ph = pair_phs[t]
                        h_sb = h_pool.tile([H_MSG, TILE], BF16, tag="h")
                        nc.scalar.activation(out=h_sb[:], in_=ph[:], func=Relu)
                        for (s0, r0, n, g) in tiles[t]:
                            if g == 1:
                                nc.vector.tensor_copy(
                                    out=hagg[:, r0:r0 + n],
                                    in_=h_sb[:, s0:s0 + n])
                            else:
                                nc.vector.tensor_reduce(
                                    out=hagg[:, r0:r0 + n],
                                    in_=h_sb[:, s0:s0 + n * g].rearrange(
                                        "p (n g) -> p n g", g=g),
                                    axis=mybir.AxisListType.X,
                                    op=mybir.AluOpType.add)
                        while (next_upd < N_UPD
                               and upd_tile.get(next_upd, T) <= t):
                            emit_upd(next_upd, pu_pool)
                            next_upd += 1
                    while next_upd < N_UPD:
                        emit_upd(next_upd, pu_pool)
                        next_upd += 1
    if not nc.is_finalized():
        nc.finalize()
    return nc


def kernel(x, edge_index, edge_features, W1, b1, W2, b2, U1, bu1, U2, bu2):
    x = np.asarray(x, dtype=np.float32)
    ei = np.asarray(edge_index).astype(np.int64)
    ef = np.asarray(edge_features, dtype=np.float32)
    src, dst = ei[0], ei[1]

    xbf = x.astype(nbf16)

    core_of = dst // N_CORE
    nl = dst - core_of * N_CORE

    # per-core degree vectors and the shared (max-over-cores) sorted profile
    degs = np.zeros((N_CORES, N_CORE), dtype=np.int64)
    np.add.at(degs, (core_of, nl), 1)
    degprof = np.sort(degs, axis=1).max(axis=0)          # ascending ranks
    degpad_full = np.concatenate([
        np.maximum(degprof, 1),
        np.ones(NPAD - N_CORE, dtype=np.int64),
    ])

    tiles, slot_base, S = _build_schedule(degpad_full)

    w1ab_h = np.ascontiguousarray(W1[:2 * D_NODE]).astype(nbf16)
    w1ca_h = np.ascontiguousarray(np.concatenate(
        [W1[2 * D_NODE:], np.asarray(b1, dtype=np.float32).reshape(1, H_MSG)],
        axis=0)).astype(nbf16)
    w2_h = np.asarray(W2).astype(nbf16)
    u1_h = np.asarray(U1).astype(nbf16)
    u2_h = np.asarray(U2).astype(nbf16)
    b2_h = np.asarray(b2, dtype=np.float32).reshape(1, D_NODE).astype(nbf16)
    bu1_h = np.asarray(bu1, dtype=np.float32).reshape(H_UPD, 1)
    bu2_h = np.asarray(bu2, dtype=np.float32).reshape(D_NODE, 1)

    in_maps = []
    perms = []
    for c in range(N_CORES):
        deg_c = degs[c]
        perm = np.argsort(deg_c, kind="stable")          # rank -> node
        perms.append(perm)
        rank_of = np.empty(N_CORE, dtype=np.int64)
        rank_of[perm] = np.arange(N_CORE)

        m = core_of == c
        e_src = src[m]
        e_dst_l = nl[m]
        e_ef = ef[m]
        key = rank_of[e_dst_l]
        order = np.argsort(key, kind="stable")
        key_s = key[order]
        first = np.searchsorted(key_s, np.arange(N_CORE), side="left")
        pos = np.arange(key_s.shape[0]) - first[key_s]
        slot = slot_base[key_s] + pos

        e_src_s = e_src[order]
        e_dst_s = e_dst_l[order] + c * N_CORE
        e_ef_s = e_ef[order]

        xsd_T = np.zeros((S, 128), dtype=nbf16)
        xsd_T[slot, 0:D_NODE] = xbf[e_src_s]
        xsd_T[slot, D_NODE:128] = xbf[e_dst_s]
        xsd_h = np.ascontiguousarray(xsd_T.T)

        eft_T = np.zeros((S, D_EDGE + 1), dtype=np.float32)
        eft_T[slot, 0:D_EDGE] = np.log1p(e_ef_s)
        eft_T[slot, D_EDGE] = 1.0
        eft_h = np.ascontiguousarray(eft_T.T.astype(nbf16))

        xt_T = np.zeros((NPAD, D_NODE), dtype=nbf16)
        xt_T[:N_CORE] = xbf[c * N_CORE + perm]
        xt_h = np.ascontiguousarray(xt_T.T)

        deg_f = np.zeros(NPAD, dtype=np.float32)
        deg_f[:N_CORE] = deg_c[perm]
        b2deg_h = np.ascontiguousarray(
            (np.asarray(b2, dtype=np.float32)[:, None]
             * deg_f[None, :]).astype(nbf16))

        in_maps.append({
            "xsd": xsd_h, "eft": eft_h, "xt": xt_h, "b2deg": b2deg_h,
            "w1ab": w1ab_h, "w1ca": w1ca_h, "w2": w2_h, "u1": u1_h,
            "u2": u2_h, "bu1c": bu1_h, "bu2c": bu2_h,
        })

    _install_trace_hook_if_possible()
    nc = _build_program(tiles, S)
    res = run_bass_kernel_spmd(nc, in_maps, list(range(N_CORES)))
    global _last_results
    _last_results = res

    out = np.empty((N_NODES, D_NODE), dtype=np.float32)
    for c in range(N_CORES):
        ot = res.results[c]["outt"].T                    # [NPAD, 64] rank order
        out[c * N_CORE + perms[c]] = ot[:N_CORE]
    return out
